# revision 52
# baseline (speedup 1.0000x reference)
"""Trainium2 Bass kernel for nn_Attention_3736621547687.

B=1, S=2048, HID=2048, NH=16, NKV=4, HD=128 attention block:
qkv proj -> per-head RMSNorm(q,k) -> RoPE -> causal GQA attention -> o proj.

Sharding: tensor-parallel over heads across 8 cores. Core c owns q heads
{2c, 2c+1} and kv head c//2 (replicated across the pair of cores sharing it).
Each core computes a partial o-projection output in bf16; the host sums the
8 partials (Megatron-style row-parallel reduce) and adds the output bias.

Device-side layout: everything is computed transposed (feature dim on
partitions, sequence on the free dim) so no on-chip transposes of
activations are needed. All GEMM operands are bf16 (f32 PSUM accumulation),
which halves HBM traffic vs f32 and enables fast weight loads (FWL).

Attention specifics:
  - scores are computed as scores^T [keys, queries]; the causal mask is an
    accumulated -1e9 bias tile added by an extra identity-lhsT matmul on the
    4 diagonal-band tiles (same math as the reference's additive mask).
  - k-side rmsnorm scale is folded into exp(): per-key rs_k values are
    computed transposed ([key,1] layout via tiny N=1 matmuls) and passed as
    the activation's per-partition scale AP.
  - q-side rmsnorm scale and the 1/sqrt(qpa) softmax scale are folded into
    the q_hat write (bias=ln(SCALE) inside the Exp that produces rs_q).
  - softmax denominators: pairwise tile sums on GpSimd, then one all-ones
    matmul both reduces over partitions and broadcasts the result.
  - softmax max-subtraction is skipped: scores are ~N(0,1) after RMSNorm so
    exp() cannot overflow; mathematically identical to the reference.
"""

import math
import numpy as np
from contextlib import ExitStack

import concourse.bass as bass
import concourse.bacc as bacc
import concourse.mybir as mybir
import concourse.tile as tile
from concourse.masks import make_identity
from concourse.bass_utils import run_bass_kernel_spmd

S = 2048
HID = 2048
NH = 16
NKV = 4
HD = 128
G = NH // NKV
SCALE = float(128.0 ** -0.5)  # query_pre_attn_scalar = 128
EPS = 1e-6
NEG = -1e9

FP32 = mybir.dt.float32
F32R = mybir.dt.float32r
BF16 = mybir.dt.bfloat16
MULT = mybir.AluOpType.mult
AF = mybir.ActivationFunctionType

N_CORES = 8
SC = 512          # chunk size (seq positions per pipeline iteration)
NCH = S // SC


def r(ap):
    return ap.bitcast(F32R)


def _patch_act_tables():
    """Force Ln and Exp onto the single combined activation-table set so the
    scalar engine never reloads tables when rms-norm and softmax interleave.
    Set ids must keep their positions, so competing sets are emptied rather
    than removed."""
    import concourse.hw_specs as hw_specs
    import concourse.bacc as bacc_mod
    orig = hw_specs.get_activation_tables

    def patched(module_arch):
        t = orig(module_arch)
        for name in ("exp_and_others", "natural_log", "exp_and_friends"):
            if name in t and "natural_log_exp_and_others" in t:
                t[name] = set()
        return t

    bacc_mod.get_activation_tables = patched


def build_nc():
    _patch_act_tables()
    nc = bacc.Bacc()

    hT = nc.dram_tensor("hT", [128, NCH, 16, SC], BF16, kind="ExternalInput")
    wts_d = [nc.dram_tensor(f"wT{i}", [128, 16, HD], BF16, kind="ExternalInput")
             for i in range(4)]
    b4 = nc.dram_tensor("b4", [128, 4], FP32, kind="ExternalInput")
    woT = nc.dram_tensor("woT", [128, 2, HID], BF16, kind="ExternalInput")
    # rope tables with the per-head norm weights folded in: qcs = qw*cos etc.
    qcs_d = nc.dram_tensor("qcs", [128, S], BF16, kind="ExternalInput")
    qss_d = nc.dram_tensor("qss", [128, S], BF16, kind="ExternalInput")
    kcs_d = nc.dram_tensor("kcs", [128, S], BF16, kind="ExternalInput")
    kss_d = nc.dram_tensor("kss", [128, S], BF16, kind="ExternalInput")
    onesbd = nc.dram_tensor("onesb", [128, 128], BF16, kind="ExternalInput")
    rswapd = nc.dram_tensor("rswap", [128, 128], F32R, kind="ExternalInput")
    maskd = nc.dram_tensor("maskb", [128, 128], BF16, kind="ExternalInput")
    # [st, partition, jc, 512]: each per-st output DMA writes 4KB
    # contiguous per partition (big descriptors; host de-swizzles)
    outp = nc.dram_tensor("outp", [16, 128, 4, SC], BF16,
                          kind="ExternalOutput")

    with ExitStack() as ctx:
        tc = ctx.enter_context(tile.TileContext(nc))

        const = ctx.enter_context(tc.tile_pool(name="const", bufs=1))
        hpool = ctx.enter_context(tc.tile_pool(name="hpool", bufs=2))
        rawp = ctx.enter_context(tc.tile_pool(name="rawp", bufs=1))
        atp = ctx.enter_context(tc.tile_pool(name="atp", bufs=1))
        wpool = ctx.enter_context(tc.tile_pool(name="wpool", bufs=3))
        vpool = ctx.enter_context(tc.tile_pool(name="vpool", bufs=1))
        ppool = ctx.enter_context(tc.tile_pool(name="ppool", bufs=2))
        dpool = ctx.enter_context(tc.tile_pool(name="dpool", bufs=2))
        opool = ctx.enter_context(tc.tile_pool(name="opool", bufs=2))
        rsp = ctx.enter_context(tc.tile_pool(name="rsp", bufs=1))

        psA = ctx.enter_context(tc.tile_pool(name="psA", bufs=1, space="PSUM"))
        psB = ctx.enter_context(tc.tile_pool(name="psB", bufs=2, space="PSUM"))
        psC = ctx.enter_context(tc.tile_pool(name="psC", bufs=3, space="PSUM"))
        psO = ctx.enter_context(tc.tile_pool(name="psO", bufs=2, space="PSUM"))

        # ---- first hidden chunk: 4 sub-DMAs on the sync queue so the
        # first qkv matmuls start as soon as the first 512KB lands -------
        hts_tiles = {}
        h0parts = []
        for p_ in range(4):
            h0p = hpool.tile([128, 4, SC], BF16, tag=f"h0p{p_}",
                             name=f"h0p{p_}", bufs=1)
            nc.sync.dma_start(out=h0p, in_=hT[:, 0, 4 * p_:4 * p_ + 4, :])
            h0parts.append(h0p)

        # ---- weights + big constants on the scalar HWDGE queue ---------
        wts = [None] * 4
        for oc in (2, 0, 1, 3):
            wt = const.tile([128, 16, HD], BF16, name=f"wts{oc}",
                            tag=f"wts{oc}")
            nc.scalar.dma_start(out=wt, in_=wts_d[oc][:, :, :])
            wts[oc] = wt
        b4s = const.tile([128, 4], FP32)
        nc.scalar.dma_start(out=b4s, in_=b4[:, :])
        woTs = const.tile([128, 2, HID], BF16)
        nc.scalar.dma_start(out=woTs, in_=woT[:, :, :])
        # rope tables on the scalar queue (k head first, used earliest)
        kcs = const.tile([128, S], BF16)
        nc.scalar.dma_start(out=kcs, in_=kcs_d[:, :])
        kss = const.tile([128, S], BF16)
        nc.scalar.dma_start(out=kss, in_=kss_d[:, :])
        qcs = const.tile([128, S], BF16)
        nc.scalar.dma_start(out=qcs, in_=qcs_d[:, :])
        qss = const.tile([128, S], BF16)
        nc.scalar.dma_start(out=qss, in_=qss_d[:, :])
        ropetab = {0: (qcs, qss), 1: (qcs, qss), None: (kcs, kss)}

        # ---- small constants on the SWDGE queue -------------------------
        ident = const.tile([128, 128], FP32)
        make_identity(nc, ident)
        identb = const.tile([128, 128], BF16)
        nc.vector.tensor_copy(identb, ident)
        # HAM warm-up: dummy matmuls open the PE clock gate while the
        # first hidden/weight DMAs stream in
        warm = const.tile([128, 256], BF16)
        nc.vector.memset(warm, 0.0)
        for _ in range(14):
            wps = psC.tile([128, SC], FP32, tag="score", name="wps")
            nc.tensor.matmul(wps[:, 0:256], lhsT=identb, rhs=warm,
                             start=True, stop=True)
        ones128b = const.tile([128, 128], BF16)
        nc.gpsimd.dma_start(out=ones128b, in_=onesbd[:, :])
        Rm = const.tile([128, 128], F32R)
        nc.gpsimd.dma_start(out=Rm, in_=rswapd[:, :])
        masktri = const.tile([128, 128], BF16)
        nc.gpsimd.dma_start(out=masktri, in_=maskd[:, :])
        onescol = const.tile([128, 1], BF16)
        nc.vector.memset(onescol, 1.0)
        epsc = const.tile([128, 1], FP32)
        nc.vector.memset(epsc, EPS)
        lnsc = const.tile([128, 1], FP32)
        nc.vector.memset(lnsc, float(math.log(SCALE)))

        rawq = [rawp.tile([128, S], F32R, tag=f"raw{i}", name=f"raw{i}")
                for i in range(3)]
        qhat = [atp.tile([128, S], BF16, tag=f"qh{i}", name=f"qh{i}")
                for i in range(2)]
        khat = atp.tile([128, S], BF16, tag="kh")
        attnT = [atp.tile([128, S], BF16, tag=f"attnT{h}", name=f"attnT{h}")
                 for h in range(2)]
        vsb = vpool.tile([128, 16, HD], BF16, tag="vsb")
        # rs_k (k-side rmsnorm scale) in transposed [key, tile] layout
        rskT = rsp.tile([128, 16], FP32, tag="rskT")
        heads = [
            (rawq[2], khat, None),       # k head: no rs multiply on write
            (rawq[0], qhat[0], 0),
            (rawq[1], qhat[1], 1),
        ]
        raw3s = {}
        LNS = float(math.log(SCALE))
        cp_n = [0]

        def osb_copy(nc, dst, src_):
            if cp_n[0] % 2 == 0:
                nc.vector.tensor_copy(dst, src_)
            else:
                nc.scalar.copy(dst, src_)
            cp_n[0] += 1

        # ================================================================
        # Software-pipelined emission: stage lag guarantees every
        # instruction's inputs are a full pipeline iteration old.
        #   iter it: proj(it) | rope+V(it-1) | attention(it-2) | oproj(it-3)
        # ================================================================
        for it in range(NCH + 3):
            # ---- stage 1: qkv projection ------------------------------
            if it < NCH:
                sc = it
                sl = bass.ts(sc, SC)
                if sc + 1 < NCH:  # prefetch next hidden chunk
                    hts_tiles[sc + 1] = hpool.tile([128, 16, SC], BF16,
                                                   tag="hts",
                                                   name=f"hts{sc + 1}")
                    nc.sync.dma_start(out=hts_tiles[sc + 1],
                                      in_=hT[:, sc + 1, :, :])
                def finish_oc(oc, ps):
                    if oc == 3:
                        raw3 = wpool.tile([128, SC], FP32, tag="raw3",
                                          bufs=2, name="raw3")
                        nc.vector.tensor_scalar_add(raw3, ps,
                                                    b4s[:, oc:oc + 1])
                        raw3s[sc] = raw3
                    else:
                        nc.vector.tensor_scalar_add(
                            rawq[oc][:, sl], ps, b4s[:, oc:oc + 1])
                if sc == 0:
                    # kt-major with 4 concurrent psum groups: each h0part
                    # DMA arrival immediately feeds all 4 head groups
                    grp = {2: psA.tile([128, SC], FP32, tag="mm",
                                       name="g2")}
                    for oc in (0, 1, 3):
                        grp[oc] = psC.tile([128, SC], FP32, tag="score",
                                           name=f"g{oc}")
                    for p_ in range(4):
                        for oc in (2, 0, 1, 3):
                            for kt in range(4 * p_, 4 * p_ + 4):
                                nc.tensor.matmul(
                                    grp[oc], lhsT=wts[oc][:, kt, :],
                                    rhs=h0parts[p_][:, kt % 4, :],
                                    start=(kt == 0), stop=(kt == 15))
                    for oc in (2, 0, 1, 3):
                        finish_oc(oc, grp[oc])
                else:
                    hts = hts_tiles.pop(sc)
                    for oc in (2, 0, 1, 3):
                        ps = psA.tile([128, SC], FP32, tag="mm")
                        for kt in range(16):
                            nc.tensor.matmul(
                                ps, lhsT=wts[oc][:, kt, :],
                                rhs=hts[:, kt, :],
                                start=(kt == 0), stop=(kt == 15))
                        finish_oc(oc, ps)

            # ---- stage 2: rmsnorm + rope + V tiles --------------------
            if 1 <= it <= NCH:
                sc = it - 1
                sl = bass.ts(sc, SC)
                for raw, dst, qi in heads:
                    cs_t, ss_t = ropetab[qi]
                    sq = wpool.tile([128, SC], BF16, tag="sq", bufs=2)
                    nc.scalar.activation(sq, raw[:, sl].bitcast(FP32),
                                         AF.Square)
                    if qi is None:
                        # k head: rs_k computed transposed [key, 1] via
                        # tiny N=1 matmuls; consumed as exp() scale AP.
                        ssT = psC.tile([128, SC], FP32, tag="score")
                        for j in range(4):
                            nc.tensor.matmul(
                                ssT[:, j:j + 1],
                                lhsT=sq[:, bass.ts(j, 128)],
                                rhs=onescol,
                                start=True, stop=True)
                        lnT = wpool.tile([128, 4], FP32, tag="lnT", bufs=2)
                        nc.scalar.activation(lnT, ssT[:, 0:4], AF.Ln,
                                             scale=1.0 / HD, bias=epsc)
                        nc.scalar.activation(rskT[:, bass.ts(sc, 4)], lnT,
                                             AF.Exp, scale=-0.5)
                    else:
                        ssum = psC.tile([128, SC], FP32, tag="score")
                        nc.tensor.matmul(ssum, lhsT=ones128b, rhs=sq,
                                         start=True, stop=True)
                        lnb = wpool.tile([128, SC], FP32, tag="lnb", bufs=2)
                        nc.scalar.activation(lnb, ssum, AF.Ln,
                                             scale=1.0 / HD, bias=epsc)
                        # rs_q with the softmax 1/sqrt(qpa) folded in
                        nc.scalar.activation(lnb, lnb, AF.Exp, scale=-0.5,
                                             bias=lnsc)
                    rtp = psC.tile([128, SC], FP32, tag="score")
                    nc.tensor.matmul(rtp, lhsT=Rm, rhs=raw[:, sl],
                                     start=True, stop=True)
                    t1 = wpool.tile([128, SC], FP32, tag="tt", bufs=3)
                    nc.vector.tensor_mul(t1, raw[:, sl], cs_t[:, sl])
                    t2 = wpool.tile([128, SC], FP32, tag="tt", bufs=3)
                    nc.vector.tensor_mul(t2, rtp, ss_t[:, sl])
                    if qi is None:
                        nc.vector.tensor_add(dst[:, sl], t1, t2)
                    else:
                        t3 = wpool.tile([128, SC], FP32, tag="tt", bufs=3)
                        nc.vector.tensor_add(t3, t1, t2)
                        nc.vector.tensor_mul(dst[:, sl], t3, lnb)
                raw3 = raw3s.pop(sc)
                for j in range(4):
                    tt = 4 * sc + j
                    vps = psC.tile([128, SC], FP32, tag="score")
                    nc.tensor.transpose(vps[:, 0:128],
                                        raw3[:, bass.ts(j, 128)], ident)
                    nc.vector.tensor_copy(vsb[:, tt, :], vps[:, 0:128])

            # ---- stage 3: attention, both heads -----------------------
            if 2 <= it <= NCH + 1:
                sc = it - 2
                sl = bass.ts(sc, SC)
                ntt = sc * 4 + 4
                LAG = 2
                ngrp = ntt // 4
                for h in range(2):
                    outps = psO.tile([128, SC], FP32, tag="attnout")
                    drep = psC.tile([128, SC], FP32, tag="score",
                                    name="drep")
                    pgroups = {}
                    pend = []  # (g, gsum) awaiting the den accumulation

                    def emit_pv(tt):
                        g2, j2 = tt // 4, tt % 4
                        qo = 128 * j2 if tt >= sc * 4 else 0
                        pgd = pgroups[g2]
                        nc.tensor.matmul(outps[:, qo:SC],
                                         lhsT=vsb[:, tt, :],
                                         rhs=pgd[:, j2, qo:SC],
                                         start=(tt == 0),
                                         stop=(tt == ntt - 1))

                    def emit_dacc(g, gsum):
                        nc.tensor.matmul(drep, lhsT=ones128b, rhs=gsum,
                                         start=(g == 0),
                                         stop=(g == ngrp - 1))

                    for tt in range(ntt):
                        g, j = tt // 4, tt % 4
                        if j == 0:
                            pgroups[g] = ppool.tile([128, 4, SC], BF16,
                                                    tag="pt", name="pg")
                        pg = pgroups[g]
                        scp = psC.tile([128, SC], FP32, tag="score")
                        diag = tt >= sc * 4
                        qo = 128 * j if diag else 0
                        if qo:
                            # fully-masked columns: skip compute, zero pg
                            # (the den adds read the whole tile)
                            nc.gpsimd.memset(pg[:, j, 0:qo], 0.0)
                        nc.tensor.matmul(
                            scp[:, qo:SC], lhsT=khat[:, bass.ts(tt, 128)],
                            rhs=qhat[h][:, sc * SC + qo:(sc + 1) * SC],
                            start=True, stop=not diag)
                        if diag:  # triangular corner: add -1e9 above diag
                            nc.tensor.matmul(
                                scp[:, qo:qo + 128], lhsT=identb,
                                rhs=masktri,
                                start=False, stop=True)
                        nc.scalar.activation(pg[:, j, qo:SC],
                                             scp[:, qo:SC], AF.Exp,
                                             scale=rskT[:, tt:tt + 1])
                        # PV lags the exp by LAG tiles so the in-order PE
                        # stream never waits on the scalar engine
                        if tt >= LAG:
                            emit_pv(tt - LAG)
                        if j == 3:
                            # pg-facing adds on DVE (frees pg fast); the
                            # group sum runs on GpSimd; the den reduce is
                            # a lagged accumulating all-ones matmul
                            ga = ppool.tile([128, SC], BF16, tag="ga",
                                            bufs=2)
                            nc.vector.tensor_add(ga, pg[:, 0, :],
                                                 pg[:, 1, :])
                            gb = ppool.tile([128, SC], BF16, tag="gb",
                                            bufs=2)
                            nc.vector.tensor_add(gb, pg[:, 2, :],
                                                 pg[:, 3, :])
                            gsum = ppool.tile([128, SC], BF16, tag="gc",
                                              bufs=2, name="gsum")
                            nc.gpsimd.tensor_add(gsum, ga, gb)
                            if pend:
                                emit_dacc(*pend.pop())
                            pend.append((g, gsum))
                    for tt in range(max(0, ntt - LAG), ntt):
                        emit_pv(tt)
                    emit_dacc(*pend.pop())
                    drec = wpool.tile([128, SC], FP32, tag="tt", bufs=3)
                    nc.vector.reciprocal_approx_fast(drec, drep)
                    nc.vector.tensor_mul(attnT[h][:, sl], outps, drec)

            # ---- stage 4: o projection --------------------------------
            if it >= 3:
                sc = it - 3
                last = sc == NCH - 1
                for st in range(4 * sc, 4 * sc + 4):
                    if last:
                        # tail: flush in half-width DMAs to overlap drain
                        for jch in range(2):
                            osb = opool.tile([128, 2, SC], BF16,
                                             tag="osbp", name="osbp")
                            for j2 in range(2):
                                jc = 2 * jch + j2
                                ops = psB.tile([128, SC], FP32, tag="omm")
                                for h in range(2):
                                    nc.tensor.matmul(
                                        ops,
                                        lhsT=attnT[h][:, bass.ts(st, 128)],
                                        rhs=woTs[:, h, bass.ts(jc, SC)],
                                        start=(h == 0), stop=(h == 1))
                                osb_copy(nc, osb[:, j2, :], ops)
                            dmae = (nc.sync, nc.scalar, nc.gpsimd)[
                                (2 * st + jch) % 3]
                            dmae.dma_start(
                                out=outp[st, :, 2 * jch:2 * jch + 2, :],
                                in_=osb)
                        continue
                    osb = opool.tile([128, 4, SC], BF16, tag="osb")
                    for jc in range(HID // SC):
                        ops = psB.tile([128, SC], FP32, tag="omm")
                        for h in range(2):
                            nc.tensor.matmul(
                                ops, lhsT=attnT[h][:, bass.ts(st, 128)],
                                rhs=woTs[:, h, bass.ts(jc, SC)],
                                start=(h == 0), stop=(h == 1))
                        osb_copy(nc, osb[:, jc, :], ops)
                    dmae = nc.sync if st % 2 == 0 else nc.scalar
                    dmae.dma_start(out=outp[st, :, :, :], in_=osb)

    nc.compile()
    return nc


def _prep_inputs(hidden_states, cos, sin, wqkv, bqkv, wo, q_norm_w, k_norm_w):
    """Host-side layout prep + per-core sharding. All device tensors are
    pre-swizzled so every DMA has long contiguous per-partition runs."""
    import ml_dtypes
    f32 = np.float32
    bf16 = ml_dtypes.bfloat16
    hTn = np.ascontiguousarray(hidden_states.reshape(S, HID).T).astype(f32)
    # [p, chunk, kt, s] chunk-major so each chunk DMA has 16KB contiguous
    # per-partition runs (big descriptors, line-rate HBM)
    hTh = np.ascontiguousarray(
        hTn.reshape(16, 128, NCH, SC).transpose(1, 2, 0, 3)).astype(bf16)
    cosT = cos.T.astype(f32)  # [64, S]
    sinT = sin.T.astype(f32)
    cs2 = np.concatenate([cosT, cosT], axis=0)          # [128, S]
    ss2 = np.concatenate([sinT, sinT], axis=0)
    qw = q_norm_w.astype(f32).reshape(128, 1)
    qws = np.concatenate([q_norm_w[64:], q_norm_w[:64]]).astype(f32).reshape(128, 1)
    kw = k_norm_w.astype(f32).reshape(128, 1)
    kws = np.concatenate([k_norm_w[64:], k_norm_w[:64]]).astype(f32).reshape(128, 1)
    qcs = np.ascontiguousarray(qw * cs2).astype(bf16)
    qss = np.ascontiguousarray(qws * ss2).astype(bf16)
    kcs = np.ascontiguousarray(kw * cs2).astype(bf16)
    kss = np.ascontiguousarray(kws * ss2).astype(bf16)
    onesb_np = np.ones((128, 128), dtype=bf16)
    rt = np.zeros((128, 128), dtype=f32)
    rt[np.arange(64) + 64, np.arange(64)] = -1.0   # R^T[d+64, d] = -1
    rt[np.arange(64), np.arange(64) + 64] = 1.0    # R^T[d-64, d] = +1
    # causal mask bias tiles for the 4 diagonal-band positions:
    # tile j masks (q_local < 128*j + k_local) with -1e9
    kl = np.arange(128)[:, None]
    ql = np.arange(128)[None, :]
    maskb = np.where(ql >= kl, 0.0, NEG).astype(bf16)  # [128, 128]

    in_maps = []
    for c in range(N_CORES):
        kvh = c // 2
        rows = list(range(2 * c * HD, (2 * c + 2) * HD))          # q0, q1
        rows += list(range(NH * HD + kvh * HD, NH * HD + (kvh + 1) * HD))  # k
        rows += list(range((NH + NKV) * HD + kvh * HD,
                           (NH + NKV) * HD + (kvh + 1) * HD))      # v
        w_c = wqkv[rows]                       # [512, HID]
        wTc = np.ascontiguousarray(w_c.T).astype(f32)   # [HID, 512]
        wTk = wTc.reshape(16, 128, 512)
        b_c = bqkv[rows].astype(f32)           # [512]
        b4c = np.ascontiguousarray(b_c.reshape(4, 128).T)  # [128, 4]
        woc = wo[:, 2 * c * HD:(2 * c + 2) * HD]  # [HID, 256]
        woTc = np.ascontiguousarray(woc.T).astype(f32)  # [256, HID]
        woTh = np.ascontiguousarray(
            woTc.reshape(2, 128, HID).transpose(1, 0, 2)).astype(bf16)
        im = {
            "hT": hTh, "b4": b4c, "woT": woTh,
            "qcs": qcs, "qss": qss, "kcs": kcs, "kss": kss,
            "onesb": onesb_np, "rswap": rt,
            "maskb": maskb,
        }
        for oc in range(4):
            im[f"wT{oc}"] = np.ascontiguousarray(
                wTk[:, :, oc * 128:(oc + 1) * 128].transpose(1, 0, 2)
            ).astype(bf16)
        in_maps.append(im)
    return in_maps


_NC_CACHE = {}


def kernel(hidden_states, cos, sin, k_cache, v_cache, mask,
           wqkv, bqkv, wo, bo, q_norm_w, k_norm_w, kv_write_indices,
           trace=False):
    hidden_states = np.asarray(hidden_states, dtype=np.float32)
    in_maps = _prep_inputs(
        np.asarray(hidden_states), np.asarray(cos), np.asarray(sin),
        np.asarray(wqkv), np.asarray(bqkv), np.asarray(wo),
        np.asarray(q_norm_w), np.asarray(k_norm_w))

    if "nc" not in _NC_CACHE:
        _NC_CACHE["nc"] = build_nc()
    nc = _NC_CACHE["nc"]

    res = run_bass_kernel_spmd(nc, in_maps, core_ids=list(range(N_CORES)),
                               trace=trace)
    out = np.zeros((16, 128, 4, SC), np.float32)
    for rmap in res.results:
        out += np.asarray(rmap["outp"], dtype=np.float32)
    # [st, r, jc, c] -> [st*128+r, jc*512+c]
    out = out.transpose(0, 1, 2, 3).reshape(16 * 128, 4 * SC)
    out = out + np.asarray(bo, dtype=np.float32)[None, :]
    if trace:
        kernel.last_results = res
    return out.reshape(1, S, HID)


# revision 53
# speedup vs baseline: 1.0022x; 1.0022x over previous
"""Trainium2 Bass kernel for nn_Attention_3736621547687.

B=1, S=2048, HID=2048, NH=16, NKV=4, HD=128 attention block:
qkv proj -> per-head RMSNorm(q,k) -> RoPE -> causal GQA attention -> o proj.

Sharding: tensor-parallel over heads across 8 cores. Core c owns q heads
{2c, 2c+1} and kv head c//2 (replicated across the pair of cores sharing it).
Each core computes a partial o-projection output in bf16; the host sums the
8 partials (Megatron-style row-parallel reduce) and adds the output bias.

Device-side layout: everything is computed transposed (feature dim on
partitions, sequence on the free dim) so no on-chip transposes of
activations are needed. All GEMM operands are bf16 (f32 PSUM accumulation),
which halves HBM traffic vs f32 and enables fast weight loads (FWL).

Attention specifics:
  - scores are computed as scores^T [keys, queries]; the causal mask is an
    accumulated -1e9 bias tile added by an extra identity-lhsT matmul on the
    4 diagonal-band tiles (same math as the reference's additive mask).
  - k-side rmsnorm scale is folded into exp(): per-key rs_k values are
    computed transposed ([key,1] layout via tiny N=1 matmuls) and passed as
    the activation's per-partition scale AP.
  - q-side rmsnorm scale and the 1/sqrt(qpa) softmax scale are folded into
    the q_hat write (bias=ln(SCALE) inside the Exp that produces rs_q).
  - softmax denominators: pairwise tile sums on GpSimd, then one all-ones
    matmul both reduces over partitions and broadcasts the result.
  - softmax max-subtraction is skipped: scores are ~N(0,1) after RMSNorm so
    exp() cannot overflow; mathematically identical to the reference.
"""

import math
import numpy as np
from contextlib import ExitStack

import concourse.bass as bass
import concourse.bacc as bacc
import concourse.mybir as mybir
import concourse.tile as tile
from concourse.masks import make_identity
from concourse.bass_utils import run_bass_kernel_spmd

S = 2048
HID = 2048
NH = 16
NKV = 4
HD = 128
G = NH // NKV
SCALE = float(128.0 ** -0.5)  # query_pre_attn_scalar = 128
EPS = 1e-6
NEG = -1e9

FP32 = mybir.dt.float32
F32R = mybir.dt.float32r
BF16 = mybir.dt.bfloat16
MULT = mybir.AluOpType.mult
AF = mybir.ActivationFunctionType

N_CORES = 8
SC = 512          # chunk size (seq positions per pipeline iteration)
NCH = S // SC


def r(ap):
    return ap.bitcast(F32R)


def _patch_act_tables():
    """Force Ln and Exp onto the single combined activation-table set so the
    scalar engine never reloads tables when rms-norm and softmax interleave.
    Set ids must keep their positions, so competing sets are emptied rather
    than removed."""
    import concourse.hw_specs as hw_specs
    import concourse.bacc as bacc_mod
    orig = hw_specs.get_activation_tables

    def patched(module_arch):
        t = orig(module_arch)
        for name in ("exp_and_others", "natural_log", "exp_and_friends"):
            if name in t and "natural_log_exp_and_others" in t:
                t[name] = set()
        return t

    bacc_mod.get_activation_tables = patched


def build_nc():
    _patch_act_tables()
    nc = bacc.Bacc()

    hT = nc.dram_tensor("hT", [128, NCH, 16, SC], BF16, kind="ExternalInput")
    wts_d = [nc.dram_tensor(f"wT{i}", [128, 16, HD], BF16, kind="ExternalInput")
             for i in range(4)]
    b4 = nc.dram_tensor("b4", [128, 4], FP32, kind="ExternalInput")
    woT = nc.dram_tensor("woT", [128, 2, HID], BF16, kind="ExternalInput")
    # rope tables with the per-head norm weights folded in: qcs = qw*cos etc.
    qcs_d = nc.dram_tensor("qcs", [128, S], BF16, kind="ExternalInput")
    qss_d = nc.dram_tensor("qss", [128, S], BF16, kind="ExternalInput")
    kcs_d = nc.dram_tensor("kcs", [128, S], BF16, kind="ExternalInput")
    kss_d = nc.dram_tensor("kss", [128, S], BF16, kind="ExternalInput")
    onesbd = nc.dram_tensor("onesb", [128, 128], BF16, kind="ExternalInput")
    rswapd = nc.dram_tensor("rswap", [128, 128], F32R, kind="ExternalInput")
    maskd = nc.dram_tensor("maskb", [128, 128], BF16, kind="ExternalInput")
    # [st, partition, jc, 512]: each per-st output DMA writes 4KB
    # contiguous per partition (big descriptors; host de-swizzles)
    outp = nc.dram_tensor("outp", [16, 128, 4, SC], BF16,
                          kind="ExternalOutput")

    with ExitStack() as ctx:
        tc = ctx.enter_context(tile.TileContext(nc))

        const = ctx.enter_context(tc.tile_pool(name="const", bufs=1))
        hpool = ctx.enter_context(tc.tile_pool(name="hpool", bufs=2))
        rawp = ctx.enter_context(tc.tile_pool(name="rawp", bufs=1))
        atp = ctx.enter_context(tc.tile_pool(name="atp", bufs=1))
        wpool = ctx.enter_context(tc.tile_pool(name="wpool", bufs=3))
        vpool = ctx.enter_context(tc.tile_pool(name="vpool", bufs=1))
        ppool = ctx.enter_context(tc.tile_pool(name="ppool", bufs=2))
        dpool = ctx.enter_context(tc.tile_pool(name="dpool", bufs=2))
        opool = ctx.enter_context(tc.tile_pool(name="opool", bufs=2))
        rsp = ctx.enter_context(tc.tile_pool(name="rsp", bufs=1))

        psA = ctx.enter_context(tc.tile_pool(name="psA", bufs=1, space="PSUM"))
        psB = ctx.enter_context(tc.tile_pool(name="psB", bufs=2, space="PSUM"))
        psC = ctx.enter_context(tc.tile_pool(name="psC", bufs=3, space="PSUM"))
        psO = ctx.enter_context(tc.tile_pool(name="psO", bufs=2, space="PSUM"))

        # ---- first hidden chunk: 4 sub-DMAs on the sync queue so the
        # first qkv matmuls start as soon as the first 512KB lands -------
        hts_tiles = {}
        h0parts = []
        for p_ in range(4):
            h0p = hpool.tile([128, 4, SC], BF16, tag=f"h0p{p_}",
                             name=f"h0p{p_}", bufs=1)
            nc.sync.dma_start(out=h0p, in_=hT[:, 0, 4 * p_:4 * p_ + 4, :])
            h0parts.append(h0p)

        # ---- weights + big constants on the scalar HWDGE queue ---------
        wts = [None] * 4
        for oc in (2, 0, 1, 3):
            wt = const.tile([128, 16, HD], BF16, name=f"wts{oc}",
                            tag=f"wts{oc}")
            nc.scalar.dma_start(out=wt, in_=wts_d[oc][:, :, :])
            wts[oc] = wt
        b4s = const.tile([128, 4], FP32)
        nc.scalar.dma_start(out=b4s, in_=b4[:, :])
        woTs = const.tile([128, 2, HID], BF16)
        nc.scalar.dma_start(out=woTs, in_=woT[:, :, :])
        # rope tables on the scalar queue (k head first, used earliest)
        kcs = const.tile([128, S], BF16)
        nc.scalar.dma_start(out=kcs, in_=kcs_d[:, :])
        kss = const.tile([128, S], BF16)
        nc.scalar.dma_start(out=kss, in_=kss_d[:, :])
        qcs = const.tile([128, S], BF16)
        nc.scalar.dma_start(out=qcs, in_=qcs_d[:, :])
        qss = const.tile([128, S], BF16)
        nc.scalar.dma_start(out=qss, in_=qss_d[:, :])
        ropetab = {0: (qcs, qss), 1: (qcs, qss), None: (kcs, kss)}

        # ---- small constants on the SWDGE queue -------------------------
        ident = const.tile([128, 128], FP32)
        make_identity(nc, ident)
        identb = const.tile([128, 128], BF16)
        nc.vector.tensor_copy(identb, ident)
        # HAM warm-up: dummy matmuls open the PE clock gate while the
        # first hidden/weight DMAs stream in
        warm = const.tile([128, 256], BF16)
        nc.vector.memset(warm, 0.0)
        for _ in range(14):
            wps = psC.tile([128, SC], FP32, tag="score", name="wps")
            nc.tensor.matmul(wps[:, 0:256], lhsT=identb, rhs=warm,
                             start=True, stop=True)
        ones128b = const.tile([128, 128], BF16)
        nc.gpsimd.dma_start(out=ones128b, in_=onesbd[:, :])
        Rm = const.tile([128, 128], F32R)
        nc.gpsimd.dma_start(out=Rm, in_=rswapd[:, :])
        masktri = const.tile([128, 128], BF16)
        nc.gpsimd.dma_start(out=masktri, in_=maskd[:, :])
        onescol = const.tile([128, 1], BF16)
        nc.vector.memset(onescol, 1.0)
        epsc = const.tile([128, 1], FP32)
        nc.vector.memset(epsc, EPS)
        lnsc = const.tile([128, 1], FP32)
        nc.vector.memset(lnsc, float(math.log(SCALE)))

        rawq = [rawp.tile([128, S], F32R, tag=f"raw{i}", name=f"raw{i}")
                for i in range(3)]
        qhat = [atp.tile([128, S], BF16, tag=f"qh{i}", name=f"qh{i}")
                for i in range(2)]
        khat = atp.tile([128, S], BF16, tag="kh")
        attnT = [atp.tile([128, S], BF16, tag=f"attnT{h}", name=f"attnT{h}")
                 for h in range(2)]
        vsb = vpool.tile([128, 16, HD], BF16, tag="vsb")
        # rs_k (k-side rmsnorm scale) in transposed [key, tile] layout
        rskT = rsp.tile([128, 16], FP32, tag="rskT")
        heads = [
            (rawq[2], khat, None),       # k head: no rs multiply on write
            (rawq[0], qhat[0], 0),
            (rawq[1], qhat[1], 1),
        ]
        raw3s = {}
        LNS = float(math.log(SCALE))
        cp_n = [0]

        def osb_copy(nc, dst, src_):
            if cp_n[0] % 2 == 0:
                nc.vector.tensor_copy(dst, src_)
            else:
                nc.scalar.copy(dst, src_)
            cp_n[0] += 1

        # ================================================================
        # Software-pipelined emission: stage lag guarantees every
        # instruction's inputs are a full pipeline iteration old.
        #   iter it: proj(it) | rope+V(it-1) | attention(it-2) | oproj(it-3)
        # ================================================================
        for it in range(NCH + 3):
            # ---- stage 1: qkv projection ------------------------------
            if it < NCH:
                sc = it
                sl = bass.ts(sc, SC)
                if sc + 1 < NCH:  # prefetch next hidden chunk
                    hts_tiles[sc + 1] = hpool.tile([128, 16, SC], BF16,
                                                   tag="hts",
                                                   name=f"hts{sc + 1}")
                    nc.sync.dma_start(out=hts_tiles[sc + 1],
                                      in_=hT[:, sc + 1, :, :])
                def finish_oc(oc, ps):
                    if oc == 3:
                        raw3 = wpool.tile([128, SC], FP32, tag="raw3",
                                          bufs=2, name="raw3")
                        nc.vector.tensor_scalar_add(raw3, ps,
                                                    b4s[:, oc:oc + 1])
                        raw3s[sc] = raw3
                    else:
                        nc.vector.tensor_scalar_add(
                            rawq[oc][:, sl], ps, b4s[:, oc:oc + 1])
                if sc == 0:
                    # kt-major with 4 concurrent psum groups: each h0part
                    # DMA arrival immediately feeds all 4 head groups
                    grp = {2: psA.tile([128, SC], FP32, tag="mm",
                                       name="g2")}
                    for oc in (0, 1, 3):
                        grp[oc] = psC.tile([128, SC], FP32, tag="score",
                                           name=f"g{oc}")
                    for p_ in range(4):
                        for oc in (2, 0, 1, 3):
                            for kt in range(4 * p_, 4 * p_ + 4):
                                nc.tensor.matmul(
                                    grp[oc], lhsT=wts[oc][:, kt, :],
                                    rhs=h0parts[p_][:, kt % 4, :],
                                    start=(kt == 0), stop=(kt == 15))
                    for oc in (2, 0, 1, 3):
                        finish_oc(oc, grp[oc])
                else:
                    hts = hts_tiles.pop(sc)
                    for oc in (2, 0, 1, 3):
                        ps = psA.tile([128, SC], FP32, tag="mm")
                        for kt in range(16):
                            nc.tensor.matmul(
                                ps, lhsT=wts[oc][:, kt, :],
                                rhs=hts[:, kt, :],
                                start=(kt == 0), stop=(kt == 15))
                        finish_oc(oc, ps)

            # ---- stage 2: rmsnorm + rope + V tiles --------------------
            if 1 <= it <= NCH:
                sc = it - 1
                sl = bass.ts(sc, SC)
                for raw, dst, qi in heads:
                    cs_t, ss_t = ropetab[qi]
                    sq = wpool.tile([128, SC], BF16, tag="sq", bufs=2)
                    nc.scalar.activation(sq, raw[:, sl].bitcast(FP32),
                                         AF.Square)
                    if qi is None:
                        # k head: rs_k computed transposed [key, 1] via
                        # tiny N=1 matmuls; consumed as exp() scale AP.
                        ssT = psC.tile([128, SC], FP32, tag="score")
                        for j in range(4):
                            nc.tensor.matmul(
                                ssT[:, j:j + 1],
                                lhsT=sq[:, bass.ts(j, 128)],
                                rhs=onescol,
                                start=True, stop=True)
                        lnT = wpool.tile([128, 4], FP32, tag="lnT", bufs=2)
                        nc.scalar.activation(lnT, ssT[:, 0:4], AF.Ln,
                                             scale=1.0 / HD, bias=epsc)
                        nc.scalar.activation(rskT[:, bass.ts(sc, 4)], lnT,
                                             AF.Exp, scale=-0.5)
                    else:
                        ssum = psC.tile([128, SC], FP32, tag="score")
                        nc.tensor.matmul(ssum, lhsT=ones128b, rhs=sq,
                                         start=True, stop=True)
                        lnb = wpool.tile([128, SC], FP32, tag="lnb", bufs=2)
                        nc.scalar.activation(lnb, ssum, AF.Ln,
                                             scale=1.0 / HD, bias=epsc)
                        # rs_q with the softmax 1/sqrt(qpa) folded in
                        nc.scalar.activation(lnb, lnb, AF.Exp, scale=-0.5,
                                             bias=lnsc)
                    rtp = psC.tile([128, SC], FP32, tag="score")
                    nc.tensor.matmul(rtp, lhsT=Rm, rhs=raw[:, sl],
                                     start=True, stop=True)
                    t1 = wpool.tile([128, SC], FP32, tag="tt", bufs=3)
                    nc.vector.tensor_mul(t1, raw[:, sl], cs_t[:, sl])
                    t2 = wpool.tile([128, SC], FP32, tag="tt", bufs=3)
                    nc.vector.tensor_mul(t2, rtp, ss_t[:, sl])
                    if qi is None:
                        nc.vector.tensor_add(dst[:, sl], t1, t2)
                    else:
                        t3 = wpool.tile([128, SC], FP32, tag="tt", bufs=3)
                        nc.vector.tensor_add(t3, t1, t2)
                        nc.vector.tensor_mul(dst[:, sl], t3, lnb)
                raw3 = raw3s.pop(sc)
                for j in range(4):
                    tt = 4 * sc + j
                    vps = psC.tile([128, SC], FP32, tag="score")
                    nc.tensor.transpose(vps[:, 0:128],
                                        raw3[:, bass.ts(j, 128)], ident)
                    nc.vector.tensor_copy(vsb[:, tt, :], vps[:, 0:128])

            # ---- stage 3: attention, both heads -----------------------
            if 2 <= it <= NCH + 1:
                sc = it - 2
                sl = bass.ts(sc, SC)
                ntt = sc * 4 + 4
                LAG = 2
                ngrp = ntt // 4
                for h in range(2):
                    outps = psO.tile([128, SC], FP32, tag="attnout")
                    drep = psC.tile([128, SC], FP32, tag="score",
                                    name="drep")
                    pgroups = {}
                    pend = []  # (g, gsum) awaiting the den accumulation

                    def emit_pv(tt):
                        g2, j2 = tt // 4, tt % 4
                        qo = 128 * j2 if tt >= sc * 4 else 0
                        pgd = pgroups[g2]
                        nc.tensor.matmul(outps[:, qo:SC],
                                         lhsT=vsb[:, tt, :],
                                         rhs=pgd[:, j2, qo:SC],
                                         start=(tt == 0),
                                         stop=(tt == ntt - 1))

                    def emit_dacc(g, gsum):
                        nc.tensor.matmul(drep, lhsT=ones128b, rhs=gsum,
                                         start=(g == 0),
                                         stop=(g == ngrp - 1))

                    for tt in range(ntt):
                        g, j = tt // 4, tt % 4
                        if j == 0:
                            pgroups[g] = ppool.tile([128, 4, SC], BF16,
                                                    tag="pt", name="pg")
                        pg = pgroups[g]
                        scp = psC.tile([128, SC], FP32, tag="score")
                        diag = tt >= sc * 4
                        qo = 128 * j if diag else 0
                        if qo:
                            # fully-masked columns: skip compute, zero pg
                            # (the den adds read the whole tile)
                            nc.gpsimd.memset(pg[:, j, 0:qo], 0.0)
                        nc.tensor.matmul(
                            scp[:, qo:SC], lhsT=khat[:, bass.ts(tt, 128)],
                            rhs=qhat[h][:, sc * SC + qo:(sc + 1) * SC],
                            start=True, stop=not diag)
                        if diag:  # triangular corner: add -1e9 above diag
                            nc.tensor.matmul(
                                scp[:, qo:qo + 128], lhsT=identb,
                                rhs=masktri,
                                start=False, stop=True)
                        nc.scalar.activation(pg[:, j, qo:SC],
                                             scp[:, qo:SC], AF.Exp,
                                             scale=rskT[:, tt:tt + 1])
                        # PV lags the exp by LAG tiles so the in-order PE
                        # stream never waits on the scalar engine
                        if tt >= LAG:
                            emit_pv(tt - LAG)
                        if j == 3:
                            # pg-facing adds on DVE (frees pg fast); the
                            # group sum runs on GpSimd; the den reduce is
                            # a lagged accumulating all-ones matmul
                            ga = ppool.tile([128, SC], BF16, tag="ga",
                                            bufs=2)
                            nc.vector.tensor_add(ga, pg[:, 0, :],
                                                 pg[:, 1, :])
                            gb = ppool.tile([128, SC], BF16, tag="gb",
                                            bufs=2)
                            nc.vector.tensor_add(gb, pg[:, 2, :],
                                                 pg[:, 3, :])
                            gsum = ppool.tile([128, SC], BF16, tag="gc",
                                              bufs=2, name="gsum")
                            nc.gpsimd.tensor_add(gsum, ga, gb)
                            if pend:
                                emit_dacc(*pend.pop())
                            pend.append((g, gsum))
                    for tt in range(max(0, ntt - LAG), ntt):
                        emit_pv(tt)
                    emit_dacc(*pend.pop())
                    drec = wpool.tile([128, SC], FP32, tag="tt", bufs=3)
                    nc.vector.reciprocal_approx_fast(drec, drep)
                    nc.vector.tensor_mul(attnT[h][:, sl], outps, drec)

            # ---- stage 4: o projection --------------------------------
            if it >= 3:
                sc = it - 3
                last = sc == NCH - 1
                for st in range(4 * sc, 4 * sc + 4):
                    if last:
                        # tail: flush in half-width DMAs to overlap drain
                        for jch in range(2):
                            osb = opool.tile([128, 2, SC], BF16,
                                             tag="osbp", name="osbp")
                            for j2 in range(2):
                                jc = 2 * jch + j2
                                ops = psB.tile([128, SC], FP32, tag="omm")
                                for h in range(2):
                                    nc.tensor.matmul(
                                        ops,
                                        lhsT=attnT[h][:, bass.ts(st, 128)],
                                        rhs=woTs[:, h, bass.ts(jc, SC)],
                                        start=(h == 0), stop=(h == 1))
                                osb_copy(nc, osb[:, j2, :], ops)
                            dmae = nc.sync if (st + jch) % 2 == 0 \
                                else nc.scalar
                            dmae.dma_start(
                                out=outp[st, :, 2 * jch:2 * jch + 2, :],
                                in_=osb)
                        continue
                    osb = opool.tile([128, 4, SC], BF16, tag="osb")
                    for jc in range(HID // SC):
                        ops = psB.tile([128, SC], FP32, tag="omm")
                        for h in range(2):
                            nc.tensor.matmul(
                                ops, lhsT=attnT[h][:, bass.ts(st, 128)],
                                rhs=woTs[:, h, bass.ts(jc, SC)],
                                start=(h == 0), stop=(h == 1))
                        osb_copy(nc, osb[:, jc, :], ops)
                    dmae = nc.sync if st % 2 == 0 else nc.scalar
                    dmae.dma_start(out=outp[st, :, :, :], in_=osb)

    nc.compile()
    return nc


def _prep_inputs(hidden_states, cos, sin, wqkv, bqkv, wo, q_norm_w, k_norm_w):
    """Host-side layout prep + per-core sharding. All device tensors are
    pre-swizzled so every DMA has long contiguous per-partition runs."""
    import ml_dtypes
    f32 = np.float32
    bf16 = ml_dtypes.bfloat16
    hTn = np.ascontiguousarray(hidden_states.reshape(S, HID).T).astype(f32)
    # [p, chunk, kt, s] chunk-major so each chunk DMA has 16KB contiguous
    # per-partition runs (big descriptors, line-rate HBM)
    hTh = np.ascontiguousarray(
        hTn.reshape(16, 128, NCH, SC).transpose(1, 2, 0, 3)).astype(bf16)
    cosT = cos.T.astype(f32)  # [64, S]
    sinT = sin.T.astype(f32)
    cs2 = np.concatenate([cosT, cosT], axis=0)          # [128, S]
    ss2 = np.concatenate([sinT, sinT], axis=0)
    qw = q_norm_w.astype(f32).reshape(128, 1)
    qws = np.concatenate([q_norm_w[64:], q_norm_w[:64]]).astype(f32).reshape(128, 1)
    kw = k_norm_w.astype(f32).reshape(128, 1)
    kws = np.concatenate([k_norm_w[64:], k_norm_w[:64]]).astype(f32).reshape(128, 1)
    qcs = np.ascontiguousarray(qw * cs2).astype(bf16)
    qss = np.ascontiguousarray(qws * ss2).astype(bf16)
    kcs = np.ascontiguousarray(kw * cs2).astype(bf16)
    kss = np.ascontiguousarray(kws * ss2).astype(bf16)
    onesb_np = np.ones((128, 128), dtype=bf16)
    rt = np.zeros((128, 128), dtype=f32)
    rt[np.arange(64) + 64, np.arange(64)] = -1.0   # R^T[d+64, d] = -1
    rt[np.arange(64), np.arange(64) + 64] = 1.0    # R^T[d-64, d] = +1
    # causal mask bias tiles for the 4 diagonal-band positions:
    # tile j masks (q_local < 128*j + k_local) with -1e9
    kl = np.arange(128)[:, None]
    ql = np.arange(128)[None, :]
    maskb = np.where(ql >= kl, 0.0, NEG).astype(bf16)  # [128, 128]

    in_maps = []
    for c in range(N_CORES):
        kvh = c // 2
        rows = list(range(2 * c * HD, (2 * c + 2) * HD))          # q0, q1
        rows += list(range(NH * HD + kvh * HD, NH * HD + (kvh + 1) * HD))  # k
        rows += list(range((NH + NKV) * HD + kvh * HD,
                           (NH + NKV) * HD + (kvh + 1) * HD))      # v
        w_c = wqkv[rows]                       # [512, HID]
        wTc = np.ascontiguousarray(w_c.T).astype(f32)   # [HID, 512]
        wTk = wTc.reshape(16, 128, 512)
        b_c = bqkv[rows].astype(f32)           # [512]
        b4c = np.ascontiguousarray(b_c.reshape(4, 128).T)  # [128, 4]
        woc = wo[:, 2 * c * HD:(2 * c + 2) * HD]  # [HID, 256]
        woTc = np.ascontiguousarray(woc.T).astype(f32)  # [256, HID]
        woTh = np.ascontiguousarray(
            woTc.reshape(2, 128, HID).transpose(1, 0, 2)).astype(bf16)
        im = {
            "hT": hTh, "b4": b4c, "woT": woTh,
            "qcs": qcs, "qss": qss, "kcs": kcs, "kss": kss,
            "onesb": onesb_np, "rswap": rt,
            "maskb": maskb,
        }
        for oc in range(4):
            im[f"wT{oc}"] = np.ascontiguousarray(
                wTk[:, :, oc * 128:(oc + 1) * 128].transpose(1, 0, 2)
            ).astype(bf16)
        in_maps.append(im)
    return in_maps


_NC_CACHE = {}


def kernel(hidden_states, cos, sin, k_cache, v_cache, mask,
           wqkv, bqkv, wo, bo, q_norm_w, k_norm_w, kv_write_indices,
           trace=False):
    hidden_states = np.asarray(hidden_states, dtype=np.float32)
    in_maps = _prep_inputs(
        np.asarray(hidden_states), np.asarray(cos), np.asarray(sin),
        np.asarray(wqkv), np.asarray(bqkv), np.asarray(wo),
        np.asarray(q_norm_w), np.asarray(k_norm_w))

    if "nc" not in _NC_CACHE:
        _NC_CACHE["nc"] = build_nc()
    nc = _NC_CACHE["nc"]

    res = run_bass_kernel_spmd(nc, in_maps, core_ids=list(range(N_CORES)),
                               trace=trace)
    out = np.zeros((16, 128, 4, SC), np.float32)
    for rmap in res.results:
        out += np.asarray(rmap["outp"], dtype=np.float32)
    # [st, r, jc, c] -> [st*128+r, jc*512+c]
    out = out.transpose(0, 1, 2, 3).reshape(16 * 128, 4 * SC)
    out = out + np.asarray(bo, dtype=np.float32)[None, :]
    if trace:
        kernel.last_results = res
    return out.reshape(1, S, HID)
